# revision 38
# baseline (speedup 1.0000x reference)
"""Single-head causal attention (B=8, T=2048, C=1024, H=64) on 8 TRN2 NeuronCores.

Strategy: pure data parallelism -- batch element b runs on core b. Each core:

    Q = q_b @ Wq ; K = k_b @ Wk ; V = k_b @ Wv          (projections)
    S = Q @ K^T / sqrt(C), causal-masked ; P = exp(S)    (no max-subtract:
    out = (P @ V) / (P @ 1)                               S is well-scaled)

Device-side layout (fp32 PSUM accumulation everywhere):
  * Inputs ship on ONE priority-ordered sync-ring DMA queue: weights, k
    block 0 (bf16), q blocks (fp8), k blocks 1-3 (fp8). q and late-k
    tolerate fp8 (they only feed scores / late-row V averages); k block 0
    stays bf16 because V rows 0:511 feed the output nearly raw. Wq and Wk
    are pre-scaled x64 on host so their fp8/bf16 encodings avoid fp8
    subnormals; the x4096 on scores is folded into the exp scale, and the
    x64 on late V blocks is folded into the V-tile evacuation.
  * A key-block-major software pipeline: projections run as k blocks
    arrive; attention cells (key tile j x 512-col i-chunk c) are emitted
    front-loaded. OUT^T accumulates per i-chunk in PSUM; chunks 0,1
    complete early and their banks recycle for chunks 2,3.
  * Q projection is column-tiled: two q-blocks project concurrently in
    the two 64-column halves of the PE array (blocks 0,3 land in
    partitions 0:64 / 1,2 in 64:128, exactly where the row-tiled scores
    want them), with interleaved emission so LDWEIGHTS pulls ahead.
  * Scores S^T (key index on partitions) have contraction H=64, so score
    matmuls run row-tiled: group A (i-chunks 0,3) on array rows 0:63 with
    operands at partitions 0:64, group B (i-chunks 1,2) on rows 64:127.
    K^T is duplicated into the upper partition half with an SBUF->SBUF
    DMA on the otherwise-idle gpsimd SWDGE queue.
  * Score cells first-fit-pack into [128, 1024] 2-bank PSUM tiles (cells
    never cross a bank) and one exp (scalar engine) covers the whole
    tile, amortizing the ~352-cycle ACTIVATE startup. The scalar engine
    is the throughput ceiling of the attention phase; the packing plus a
    3-deep shared PSUM pool keeps it saturated.
  * P^T tiles (bf16, SBUF) are directly the moving operand of the P @ V
    accumulation; softmax denominators come free via a ones column
    prepended to V (row 0 of the accumulator is P @ 1). V natural tiles
    come from PE transposes (4 per block into one PSUM tile, one copy).
  * Normalization: reciprocal of row 0, gpsimd partition-broadcast, DVE
    multiply, store on the sync ring (the scalar ring would head-of-line
    block exps). Device emits out^T [H, T]; host transposes.
  * 6 warmup matmuls on weight data at kernel start hold the PE busy
    through one HAM activity window so real matmuls run at 2.4 GHz, and a
    dummy exp preloads the ACT spline tables off the critical path.
"""

import numpy as np
import ml_dtypes

B, T, C, H = 8, 2048, 1024, 64
P = 128                  # SBUF partitions
CCH = C // P             # 8 contraction chunks
NJ = T // P              # 16 key tiles of 128
NB = T // 512            # 4 column blocks of 512
WS = 64.0                # host pre-scale on Wq/Wk (+ late Wv)
SCALE = float(C) ** -0.5 / (WS * WS)

_cached = {}


def _build():
    import concourse.bass as bass
    import concourse.mybir as mybir
    import concourse.tile as tile
    from concourse import bacc

    dt = mybir.dt
    nc = bacc.Bacc("TRN2", target_bir_lowering=False, debug=False, num_devices=B)

    qT = nc.dram_tensor("qT", [NB, P, CCH, 512], dt.float8e4, kind="ExternalInput").ap()
    kT0 = nc.dram_tensor("kT0", [P, CCH, 512], dt.bfloat16, kind="ExternalInput").ap()
    kT8 = nc.dram_tensor("kT8", [3, P, CCH, 512], dt.float8e4, kind="ExternalInput").ap()
    wq = nc.dram_tensor("wq", [P, CCH, H], dt.float8e4, kind="ExternalInput").ap()
    wkvb = nc.dram_tensor("wkvb", [P, CCH, 2 * H], dt.bfloat16, kind="ExternalInput").ap()
    wkv8 = nc.dram_tensor("wkv8", [P, CCH, 2 * H], dt.float8e4, kind="ExternalInput").ap()
    dmask = nc.dram_tensor("dmask", [P, P], dt.bfloat16, kind="ExternalInput").ap()
    idb = nc.dram_tensor("idb", [P, P], dt.bfloat16, kind="ExternalInput").ap()
    out_t = nc.dram_tensor("out_t", [H, T], dt.float32, kind="ExternalOutput").ap()

    EXP = mybir.ActivationFunctionType.Exp
    MUL = mybir.AluOpType.mult

    with tile.TileContext(nc) as tc:
        with (
            tc.tile_pool(name="consts", bufs=1) as consts,
            tc.tile_pool(name="inbuf", bufs=1) as inbuf,
            tc.tile_pool(name="proj", bufs=1) as proj,
            tc.tile_pool(name="wpsum", bufs=3, space="PSUM") as wpsum,
            tc.tile_pool(name="opsum", bufs=2, space="PSUM") as opsum,
            tc.tile_pool(name="pbuf", bufs=3) as pbuf,
            tc.tile_pool(name="ebuf", bufs=2) as ebuf,
            tc.tile_pool(name="obuf", bufs=2) as obuf,
        ):
            mask_s = consts.tile([P, P], dt.bfloat16)
            idb_s = consts.tile([P, P], dt.bfloat16)
            wq_s = consts.tile([P, CCH, H], dt.float8e4)
            wkvb_s = consts.tile([P, CCH, 2 * H], dt.bfloat16)
            wkv8_s = consts.tile([P, CCH, 2 * H], dt.float8e4)
            kT0_s = inbuf.tile([P, CCH, 512], dt.bfloat16)
            kT8_s = inbuf.tile([P, 3, CCH, 512], dt.float8e4)
            qT_s = inbuf.tile([P, NB, CCH, 512], dt.float8e4)

            # ---- ONE priority-ordered input queue (sync HWDGE ring) --------
            nc.sync.dma_start(out=wkvb_s[:], in_=wkvb[:])
            nc.sync.dma_start(out=wkv8_s[:], in_=wkv8[:])
            nc.sync.dma_start(out=wq_s[:], in_=wq[:])
            nc.sync.dma_start(out=mask_s[:], in_=dmask[:])
            nc.sync.dma_start(out=idb_s[:], in_=idb[:])
            nc.sync.dma_start(out=kT0_s[:, 0:4], in_=kT0[:, 0:4])
            nc.sync.dma_start(out=kT0_s[:, 4:8], in_=kT0[:, 4:8])
            nc.sync.dma_start(out=qT_s[:, 0], in_=qT[0])
            nc.sync.dma_start(out=qT_s[:, 1], in_=qT[1])
            nc.sync.dma_start(out=kT8_s[:, 0], in_=kT8[0])
            nc.sync.dma_start(out=qT_s[:, 3], in_=qT[3])
            nc.sync.dma_start(out=qT_s[:, 2], in_=qT[2])
            nc.sync.dma_start(out=kT8_s[:, 1], in_=kT8[1])
            nc.sync.dma_start(out=kT8_s[:, 2], in_=kT8[2])

            # ---- persistent SBUF -------------------------------------------
            KVT_s = proj.tile([P, T], dt.bfloat16)   # 0:64 K^T | 64:128 V^T
            KTB_s = proj.tile([P, T], dt.bfloat16)   # 64:128 = K^T copy (grp B)
            QT_s = proj.tile([P, 2, 512], dt.bfloat16)
            V1_s = proj.tile([P, NJ, 66], dt.bfloat16)
            nc.vector.memset(V1_s[:, :, 0:1], 1.0)

            # ---- ACT table preload + PE HAM warmup -------------------------
            # warmup matmuls on a memset tile: no DMA dependency, so they
            # start right after the preamble and bridge into the first real
            # matmuls with no gap (a gap restarts the HAM activity window)
            wrmb = proj.tile([P, 512], dt.bfloat16)
            nc.vector.memset(wrmb[:], 0.25)
            wrm = ebuf.tile([1, 16], dt.float32, tag="wa")
            nc.scalar.activation(out=wrm[:], in_=wrmb[0:1, 0:16], func=EXP,
                                 scale=1.0)
            warm = wpsum.tile([P, 512], dt.float32, tag="s")
            for i in range(18):
                nc.tensor.matmul(warm[:], lhsT=wrmb[:, 0:128],
                                 rhs=wrmb[:], start=True, stop=True)

            # ---- projections ------------------------------------------------
            def kv_proj(tb):
                sl = slice(512 * tb, 512 * (tb + 1))
                KVTp = wpsum.tile([P, 512], dt.float32, tag="s")
                # col-split: K into array col-strips 0:1, V into 2:3 -- the
                # two streams run concurrently and their LDWEIGHTS pull ahead
                # (no col-group conflict), unlike one fused full-width matmul
                wsrc = wkvb_s if tb == 0 else wkv8_s
                for c in range(CCH):
                    if tb == 0:
                        rhs = kT0_s[:, c, :]
                    else:
                        rhs = kT8_s[:, tb - 1, c, :]
                    nc.tensor.matmul(KVTp[0:64, :], lhsT=wsrc[:, c, 0:64],
                                     rhs=rhs,
                                     start=(c == 0), stop=(c == CCH - 1))
                    nc.tensor.matmul(KVTp[64:128, :], lhsT=wsrc[:, c, 64:128],
                                     rhs=rhs,
                                     start=(c == 0), stop=(c == CCH - 1))
                # K-half evacuates first so the KTB duplicate (and group-A
                # scores) release before the V-half/transposes
                nc.vector.tensor_copy(out=KVT_s[0:64, sl], in_=KVTp[0:64, :])
                nc.gpsimd.dma_start(out=KTB_s[64:128, sl], in_=KVT_s[0:64, sl])
                nc.vector.tensor_copy(out=KVT_s[64:128, sl],
                                      in_=KVTp[64:128, :])
                # V natural tiles: 4 PE transposes into one PSUM tile, 1 copy
                vtp = wpsum.tile([P, 4, H], dt.bfloat16, tag="s")
                for jj in range(4):
                    j = 4 * tb + jj
                    nc.tensor.transpose(vtp[:, jj, :],
                                        KVT_s[64:128, P * j:P * (j + 1)],
                                        idb_s[64:128, 64:128])
                if tb == 0:
                    nc.vector.tensor_copy(out=V1_s[:, 0:4, 1:65], in_=vtp[:])
                else:
                    # late V blocks carry the x64 of Wv; rescale on evacuation
                    nc.vector.tensor_scalar(
                        out=V1_s[:, 4 * tb:4 * tb + 4, 1:65], in0=vtp[:],
                        scalar1=1.0 / WS, scalar2=None, op0=MUL)

            def q_proj(pair):
                # col-tiled pair: two q blocks concurrently in the two 64-col
                # array halves, interleaved so LDWEIGHTS pulls ahead
                blo, bhi = (0, 1) if pair == 0 else (3, 2)
                QTp = wpsum.tile([P, 512], dt.float32, tag="s")
                for c in range(CCH):
                    nc.tensor.matmul(QTp[0:64, :], lhsT=wq_s[:, c, :],
                                     rhs=qT_s[:, blo, c, :],
                                     start=(c == 0), stop=(c == CCH - 1))
                    nc.tensor.matmul(QTp[64:128, :], lhsT=wq_s[:, c, :],
                                     rhs=qT_s[:, bhi, c, :],
                                     start=(c == 0), stop=(c == CCH - 1))
                nc.vector.tensor_copy(out=QT_s[:, pair, :], in_=QTp[:])

            # ---- attention cells -------------------------------------------
            GRP = {0: "A", 1: "B", 2: "B", 3: "A"}
            QSLOT = {0: (slice(0, 64), 0), 1: (slice(64, 128), 0),
                     2: (slice(64, 128), 1), 3: (slice(0, 64), 1)}
            NPV = {0: 4, 1: 8, 2: 12, 3: 16}
            pend = []          # (j, c, w, off) cells in the open stile
            cur = {"stile": None}
            pv_done = {0: 0, 1: 0, 2: 0, 3: 0}
            OUT = {}

            def flush():
                cells = list(pend)
                if not cells:
                    return
                pend.clear()
                stile = cur["stile"]
                cur["stile"] = None
                end = max(off + w for _, _, w, off in cells)
                Pt = pbuf.tile([P, 1024], dt.bfloat16, tag="pt")
                nc.scalar.activation(out=Pt[:, 0:end], in_=stile[:, 0:end],
                                     func=EXP, scale=SCALE)
                for j, c, w, off in cells:
                    if j // 4 == c:  # diagonal: zero strictly-upper triangle
                        nc.vector.tensor_mul(Pt[:, off:off + P],
                                             Pt[:, off:off + P], mask_s[:])
                for j, c, w, off in cells:
                    lo = max(512 * c, P * j)
                    nc.tensor.matmul(OUT[c][:, lo - 512 * c:512],
                                     lhsT=V1_s[:, j, 0:65],
                                     rhs=Pt[:, off:off + w],
                                     start=(pv_done[c] == 0),
                                     stop=(pv_done[c] == NPV[c] - 1))
                    pv_done[c] += 1

            def fit(w):
                """First-fit offset for a w-wide cell; cells can't cross the
                fp32 PSUM bank boundary at column 512."""
                ends = [0, 512]
                for _, _, cw, coff in pend:
                    b = coff // 512
                    ends[b] = max(ends[b], coff + cw)
                for b in range(2):
                    if ends[b] + w <= 512 * (b + 1):
                        return ends[b]
                return None

            def cell(j, c):
                lo = max(512 * c, P * j)
                w = 512 * (c + 1) - lo
                if c not in OUT:
                    OUT[c] = opsum.tile([H + 1, 512], dt.float32, tag="o",
                                        name=f"out{c}")
                off = fit(w)
                if off is None:
                    flush()
                    off = 0
                if cur["stile"] is None:
                    cur["stile"] = wpsum.tile([P, 1024], dt.float32, tag="s",
                                              name="stile")
                rows, slot = QSLOT[c]
                if GRP[c] == "A":
                    lhsT = KVT_s[0:64, P * j:P * (j + 1)]
                else:
                    lhsT = KTB_s[64:128, P * j:P * (j + 1)]
                nc.tensor.matmul(cur["stile"][:, off:off + w], lhsT=lhsT,
                                 rhs=QT_s[rows, slot, lo - 512 * c:512],
                                 start=True, stop=True)
                pend.append((j, c, w, off))

            def norm(c):
                flush()
                linv = ebuf.tile([1, 512], dt.float32, tag="l")
                nc.vector.reciprocal_approx_fast(linv[:], OUT[c][0:1, :])
                lb = ebuf.tile([H + 1, 512], dt.float32, tag="b")
                nc.gpsimd.partition_broadcast(lb[:], linv[:])
                ot = obuf.tile([H + 1, 512], dt.float32, tag="ot")
                nc.vector.tensor_mul(ot[:], OUT[c][:], lb[:])
                nc.sync.dma_start(out=out_t[:, 512 * c:512 * (c + 1)],
                                  in_=ot[1:H + 1, :])
                del OUT[c]

            # ---- schedule ---------------------------------------------------
            kv_proj(0)
            q_proj(0)
            # block-0 A cells first (KTB0 needs an extra SWDGE hop)
            for j in range(4):
                cell(j, 0)
            for j in range(4):
                cell(j, 1)
            norm(0)
            kv_proj(1)
            for j in range(4, 8):
                cell(j, 1)
            norm(1)
            q_proj(1)
            for j in range(4):
                cell(j, 3); cell(j, 2)
            kv_proj(2)
            for j in range(4, 8):
                cell(j, 3); cell(j, 2)
            for j in range(8, 12):
                cell(j, 3); cell(j, 2)
            norm(2)
            kv_proj(3)
            for j in range(12, 16):
                cell(j, 3)
            norm(3)

    nc.compile()
    return nc


def _get_nc():
    if "nc" not in _cached:
        _cached["nc"] = _build()
    return _cached["nc"]


def _block(xT):
    """[C, T] -> [NB, P, CCH, 512] so each 512-col block is contiguous."""
    return np.ascontiguousarray(
        xT.reshape(CCH, P, NB, 512).transpose(2, 1, 0, 3))


def _wblock(w):
    """[C, Hw] -> [P, CCH, Hw] contiguous (contraction chunks on partitions)."""
    return np.ascontiguousarray(
        w.reshape(CCH, P, w.shape[1]).transpose(1, 0, 2))


def _host_inputs(q, k, Wq, Wk, Wv):
    bf16 = ml_dtypes.bfloat16
    f8 = ml_dtypes.float8_e4m3
    wq_h = _wblock((Wq * WS).astype(f8))
    wkvb_h = _wblock(np.concatenate([Wk * WS, Wv], axis=1).astype(bf16))
    wkv8_h = _wblock(np.concatenate([Wk * WS, Wv * WS], axis=1).astype(f8))
    dmask_h = np.triu(np.ones((P, P), dtype=np.float32)).astype(bf16)
    idb_h = np.eye(P, dtype=np.float32).astype(bf16)
    in_maps = []
    for b in range(B):
        kb = _block(k[b].T)
        in_maps.append({
            "qT": _block(q[b].T.astype(f8)),
            "kT0": kb[0].astype(bf16),
            "kT8": kb[1:4].astype(f8),
            "wq": wq_h,
            "wkvb": wkvb_h,
            "wkv8": wkv8_h,
            "dmask": dmask_h,
            "idb": idb_h,
        })
    return in_maps


def kernel(q, k, Wq, Wk, Wv):
    from concourse.bass_utils import run_bass_kernel_spmd

    nc = _get_nc()
    in_maps = _host_inputs(q, k, Wq, Wk, Wv)
    res = run_bass_kernel_spmd(nc, in_maps, list(range(B)))
    return np.stack(
        [res.results[b]["out_t"].T for b in range(B)]).astype(np.float32)


if __name__ == "__main__":
    rng = np.random.default_rng(0)
    q = rng.standard_normal((B, T, C)).astype(np.float32)
    k = rng.standard_normal((B, T, C)).astype(np.float32)
    Wq = (rng.standard_normal((C, H)) * 0.02).astype(np.float32)
    Wk = (rng.standard_normal((C, H)) * 0.02).astype(np.float32)
    Wv = (rng.standard_normal((C, H)) * 0.02).astype(np.float32)
    o = kernel(q, k, Wq, Wk, Wv)
    print("out", o.shape, o.dtype, float(np.abs(o).max()))
